# revision 5
# baseline (speedup 1.0000x reference)
"""Trainium2 Bass kernel for nn_DotProductAttention_11433202942822.

Math (per (b, h) pair, T=2048, D=64):
    S = Q @ K^T * (1/sqrt(64))            [T1, T2]
    attn = softmax(S, axis=T1)            <- softmax over the QUERY axis
    out = attn @ V                        [T1, D]

Structure (per core, 4 (b,h) pairs):
  * mm1: S^T = K @ Q^T tiles [128 k2, T q] in PSUM, fp16 inputs, 2x64-row
    PE row-group pairing (even k2-tiles on partitions 0-63, odd on 64-127).
  * exp split across TWO engines (the ScalarE LUT is the bottleneck
    resource at 1 elem/cycle/lane):
      - ScalarE: activation(Exp) with fused accum_out on 1536 of each
        tile's 2048 columns.
      - VectorE: Schraudolph fp16 exp on the last 512 columns (its own
        PSUM bank so the engines never share a bank): a tensor_scalar
        affine fp32->int16 written through a bitcast view of the fp16 et
        tile; code = round(S*scale*log2e*1024) + (15*1024 - 45). Applied
        to 1/4 of the elements -> ~8e-3 output rel err (gate 2e-2).
      - DVE-part colsum: tensor_scalar copy with accum_out over those
        512 columns; partials combined with the ACT accumulators.
  * PE HAM clock-gate management (worth ~30us): a 20-matmul full-array
    warm-up burst at kernel start plus 8-matmul re-warm bursts in each
    bh-boundary bubble (the out banks are free right after evacuation).
  * normalization folded into V: vp = V * (1/colsum) per k2-tile (fp16).
  * mm2: out^T += vp.T @ et with 2x64-col PE col-group pairing, 6 tiles
    deep behind the exp stream (8 et buffers), drained eagerly at the
    end of the last bh.

Sharding: batch*heads = 32 pairs, 4 per core across 8 cores (head/data
parallel, no cross-core communication).
"""

import sys

import numpy as np

if "/opt/trn_rl_repo" not in sys.path:
    sys.path.insert(0, "/opt/trn_rl_repo")

import concourse.tile as tile  # noqa: E402
from concourse import bacc, mybir  # noqa: E402
from concourse.bass_utils import run_bass_kernel_spmd  # noqa: E402

P = 128
D = 64
SCALE = 1.0 / (D ** 0.5)
N_CORES = 8

F32 = mybir.dt.float32
F16 = mybir.dt.float16
I16 = mybir.dt.int16

LOG2E = 1.4426950408889634
# Schraudolph fp16: code = trunc(S * AFF_A + AFF_B); bitcast fp16 ~ exp(S*SCALE)
AFF_A = SCALE * LOG2E * 1024.0
AFF_B = 15.0 * 1024.0 - 45.0  # HW convert rounds to nearest (probe-verified)

# Columns per TILE handled by the DVE Schraudolph path (rest on ACT).
# Must be a whole number of PSUM banks (512) so ScalarE and VectorE never
# read the same PSUM bank concurrently.
W_DVE = 512


def build_attention_nc(BH: int, T: int, debug: bool = False):
    """Per-core Bass module. See module docstring for layouts."""
    assert T % 1024 == 0 and T % P == 0
    KT_TILES = T // P
    CHUNK = 1024  # PSUM score-chunk (2 banks)

    nc = bacc.Bacc("TRN2", target_bir_lowering=False, debug=debug)

    qt = nc.dram_tensor("qt", [BH, 2 * D, T], F16, kind="ExternalInput").ap()
    kt = nc.dram_tensor("kt", [BH, 2 * D, T // 2], F16, kind="ExternalInput").ap()
    v = nc.dram_tensor("v", [BH, P, T // P, D], F32, kind="ExternalInput").ap()
    out = nc.dram_tensor("out", [BH, D, T], F32, kind="ExternalOutput").ap()

    with tile.TileContext(nc) as tc:
        with (
            tc.tile_pool(name="ins", bufs=1) as ins_pool,
            tc.tile_pool(name="et", bufs=8) as et_pool,
            tc.tile_pool(name="small", bufs=8) as small_pool,
            tc.tile_pool(name="osb", bufs=2) as osb_pool,
            tc.tile_pool(name="spsum", bufs=3, space="PSUM") as s_pool,
            tc.tile_pool(name="opsum", bufs=1, space="PSUM") as o_pool,
        ):
            qt_sb = ins_pool.tile([2 * D, BH, T], F16, tag="qt_sb")
            kt_sb = ins_pool.tile([2 * D, BH, T // 2], F16, tag="kt_sb")
            v_sb = ins_pool.tile([P, BH, KT_TILES, D], F32, tag="v_sb")
            # scratch sink for the DVE-part colsum pass
            sink = ins_pool.tile([P, W_DVE], F16, tag="sink")
            # Warm the ACT exp table during input DMAs.
            warm = small_pool.tile([P, 1], F32, tag="warm")
            nc.vector.memset(warm[:], 0.0)
            nc.scalar.activation(
                warm[:], warm[:], mybir.ActivationFunctionType.Exp
            )
            # Warm the PE HAM clock gate during the input DMAs: ~7us of
            # dummy matmuls (no DMA dependency) cross the 4096-cycle SHORT
            # window so the real mm1 stream starts at 2.4 GHz.
            wdum = ins_pool.tile([P, 512], F16, tag="wdum")
            nc.vector.memset(wdum[:], 0.0)
            warm_ps = o_pool.tile([2 * D, T // 2], F32, tag="out_ps")
            for _ in range(20):
                nc.tensor.matmul(
                    warm_ps[:, 0:512],
                    lhsT=wdum[:, 0:128],
                    rhs=wdum[:],
                    start=True,
                    stop=True,
                )

            for bh in range(BH):
                nc.sync.dma_start(qt_sb[:, bh, :], qt[bh])
                nc.sync.dma_start(kt_sb[:, bh, :], kt[bh])
                nc.sync.dma_start(v_sb[:, bh], v[bh])

            def emit_mm2(out_ps, vp, et, t):
                for c in range(0, T, 512):
                    half = c // (T // 2)
                    qh = c % (T // 2)
                    nc.tensor.matmul(
                        out_ps[half * D:(half + 1) * D, qh:qh + 512],
                        lhsT=vp[:],
                        rhs=et[:, c:c + 512],
                        start=(t == 0),
                        stop=(t == KT_TILES - 1),
                        skip_group_check=True,
                    )

            def emit_norm_vp(bh, t, et, partial_sums):
                """Combine colsum partials + reciprocal + scale V."""
                while len(partial_sums) > 1:
                    stot = small_pool.tile([P, 1], F32, tag="stot")
                    nc.vector.tensor_add(
                        stot[:], partial_sums[0][:], partial_sums[1][:]
                    )
                    partial_sums = [stot] + partial_sums[2:]
                rec = small_pool.tile([P, 1], F32, tag="rec")
                nc.vector.reciprocal(rec[:], partial_sums[0][:])
                vp = small_pool.tile([P, D], F16, tag="vp")
                nc.vector.tensor_scalar_mul(vp[:], v_sb[:, bh, t, :], rec[:])
                return vp

            def evacuate(bh, out_ps):
                osb = osb_pool.tile([2 * D, T // 2], F32, tag="osb")
                nc.vector.tensor_copy(osb[:], out_ps[:])
                nc.sync.dma_start(out[bh][:, 0:T // 2], osb[0:D])
                nc.sync.dma_start(out[bh][:, T // 2:T], osb[D:2 * D])
                if bh < BH - 1:
                    # keep the PE HAM warm through the boundary bubble: the
                    # out banks are free between the copy above and the
                    # next bh's first accumulating matmul
                    rw = o_pool.tile([2 * D, T // 2], F32, tag="out_ps")
                    for _ in range(8):
                        nc.tensor.matmul(
                            rw[:, 0:512],
                            lhsT=wdum[:, 0:128],
                            rhs=wdum[:],
                            start=True,
                            stop=True,
                        )

            def pop_mm2(pending):
                bh_, out_ps_, vp_, et_, t_ = pending.pop(0)
                emit_mm2(out_ps_, vp_, et_, t_)
                if t_ == KT_TILES - 1:
                    evacuate(bh_, out_ps_)

            pending_mm2 = []
            for bh in range(BH):
                out_ps = o_pool.tile([2 * D, T // 2], F32, tag="out_ps")
                for j in range(KT_TILES // 2):
                    tA, tB = 2 * j, 2 * j + 1
                    etA = et_pool.tile([P, T], F16, tag="et", name="etA")
                    etB = et_pool.tile([P, T], F16, tag="et", name="etB")
                    lhsA = kt_sb[0:D, bh, j * P:(j + 1) * P]
                    lhsB = kt_sb[D:2 * D, bh, j * P:(j + 1) * P]
                    psums = {tA: [], tB: []}
                    for q0 in range(0, T, CHUNK):
                        spA = s_pool.tile([P, CHUNK], F32, tag="sp",
                                          name="spA")
                        spB = s_pool.tile([P, CHUNK], F32, tag="sp",
                                          name="spB")
                        for c in range(0, CHUNK, 512):
                            nc.tensor.matmul(
                                spA[:, c:c + 512],
                                lhsT=lhsA,
                                rhs=qt_sb[0:D, bh, q0 + c:q0 + c + 512],
                                start=True,
                                stop=True,
                            )
                            nc.tensor.matmul(
                                spB[:, c:c + 512],
                                lhsT=lhsB,
                                rhs=qt_sb[D:2 * D, bh, q0 + c:q0 + c + 512],
                                start=True,
                                stop=True,
                            )
                        # exp: last W_DVE cols of the LAST chunk go to the
                        # DVE Schraudolph path (own PSUM bank); rest -> ACT
                        # with fused accum (colsum partials for free).
                        last = q0 + CHUNK == T
                        na = CHUNK - W_DVE if last else CHUNK
                        for t, et, sp in ((tA, etA, spA), (tB, etB, spB)):
                            acc = small_pool.tile([P, 1], F32, tag="acc")
                            nc.scalar.activation(
                                et[:, q0:q0 + na],
                                sp[:, 0:na],
                                mybir.ActivationFunctionType.Exp,
                                scale=SCALE,
                                accum_out=acc[:],
                            )
                            psums[t].append(acc)
                            if last and W_DVE:
                                nc.vector.tensor_scalar(
                                    et[:, q0 + na:q0 + CHUNK].bitcast(I16),
                                    sp[:, na:CHUNK],
                                    AFF_A,
                                    AFF_B,
                                    mybir.AluOpType.mult,
                                    mybir.AluOpType.add,
                                )
                                dacc = small_pool.tile([P, 1], F32, tag="dacc")
                                nc.vector.tensor_scalar(
                                    sink[:, 0:W_DVE],
                                    et[:, q0 + na:q0 + CHUNK],
                                    1.0,
                                    0.0,
                                    mybir.AluOpType.mult,
                                    mybir.AluOpType.add,
                                    accum_out=dacc[:],
                                )
                                psums[t].append(dacc)
                    for t, et in ((tA, etA), (tB, etB)):
                        vp = emit_norm_vp(bh, t, et, psums[t])
                        pending_mm2.append((bh, out_ps, vp, et, t))
                    # deep pipeline in steady state; drain eagerly near the
                    # end so the final mm2/evacuation tail is short
                    depth = 6 if (bh < BH - 1 or j < KT_TILES // 2 - 4) else 1
                    while len(pending_mm2) > depth:
                        pop_mm2(pending_mm2)
            while pending_mm2:
                pop_mm2(pending_mm2)

    nc.compile()
    return nc


_NC_CACHE: dict = {}

TRACE = False
LAST_RESULTS = None


def _get_nc(BH: int, T: int):
    key = (BH, T)
    if key not in _NC_CACHE:
        _NC_CACHE[key] = build_attention_nc(BH, T)
    return _NC_CACHE[key]


def _reference_numpy(Q, K, V, padding_mask, isCausal):
    """Fallback exactly mirroring reference.py (never hit for spec inputs)."""
    Q = Q.astype(np.float64)
    K = K.astype(np.float64)
    V = V.astype(np.float64)
    scores = np.einsum("bhqd,bhkd->bhqk", Q, K) * SCALE
    T1 = scores.shape[2]
    mask = padding_mask[:, None, :, :].astype(np.float64)
    if isCausal:
        mask = mask * np.tril(np.ones((T1, T1)))
    scores = np.where(mask == 0, -np.inf, scores)
    m = np.max(scores, axis=2, keepdims=True)
    e = np.exp(scores - m)
    attn = e / np.sum(e, axis=2, keepdims=True)
    return np.einsum("bhqk,bhkd->bhqd", attn, V).astype(np.float32)


def kernel(Q, K, V, padding_mask, isCausal, **_unused):
    Q = np.asarray(Q, dtype=np.float32)
    K = np.asarray(K, dtype=np.float32)
    V = np.asarray(V, dtype=np.float32)
    padding_mask = np.asarray(padding_mask)
    causal = int(np.asarray(isCausal))

    B, H, T, Dd = Q.shape
    assert Dd == D
    if causal != 0 or padding_mask.min() != 1.0 or padding_mask.max() != 1.0:
        return _reference_numpy(Q, K, V, padding_mask, causal)

    BHT = B * H
    assert BHT % N_CORES == 0
    BH = BHT // N_CORES

    nc = _get_nc(BH, T)

    Qf = Q.reshape(BHT, T, D)
    Kf = K.reshape(BHT, T, D)
    Vf = V.reshape(BHT, T, D)

    QT = Qf.transpose(0, 2, 1).astype(np.float16)
    qt_all = np.ascontiguousarray(np.concatenate([QT, QT], axis=1))
    KT = Kf.transpose(0, 2, 1).astype(np.float16)
    KT4 = KT.reshape(BHT, D, T // 128, 128)
    kt_all = np.ascontiguousarray(
        np.concatenate(
            [
                KT4[:, :, 0::2, :].reshape(BHT, D, T // 2),
                KT4[:, :, 1::2, :].reshape(BHT, D, T // 2),
            ],
            axis=1,
        )
    )
    v_all = np.ascontiguousarray(
        Vf.reshape(BHT, T // P, P, D).transpose(0, 2, 1, 3)
    )

    in_maps = []
    for c in range(N_CORES):
        sl = slice(c * BH, (c + 1) * BH)
        in_maps.append(
            {
                "qt": np.ascontiguousarray(qt_all[sl]),
                "kt": np.ascontiguousarray(kt_all[sl]),
                "v": np.ascontiguousarray(v_all[sl]),
            }
        )

    res = None
    last_err = None
    for attempt in range(3):
        try:
            res = run_bass_kernel_spmd(
                nc, in_maps, core_ids=list(range(N_CORES)), trace=TRACE
            )
            break
        except Exception as e:
            last_err = e
            import time as _time

            _time.sleep(2.0)
    if res is None:
        raise last_err
    global LAST_RESULTS
    LAST_RESULTS = res

    outs = [res.results[c]["out"] for c in range(N_CORES)]
    out_all = np.concatenate(outs, axis=0)
    out = out_all.transpose(0, 2, 1).reshape(B, H, T, D)
    return np.ascontiguousarray(out).astype(np.float32)


# revision 6
# speedup vs baseline: 1.0309x; 1.0309x over previous
"""Trainium2 Bass kernel for nn_DotProductAttention_11433202942822.

Math (per (b, h) pair, T=2048, D=64):
    S = Q @ K^T * (1/sqrt(64))            [T1, T2]
    attn = softmax(S, axis=T1)            <- softmax over the QUERY axis
    out = attn @ V                        [T1, D]

Structure (per core, 4 (b,h) pairs):
  * mm1: S^T = K @ Q^T tiles [128 k2, T q] in PSUM, fp16 inputs, 2x64-row
    PE row-group pairing (even k2-tiles on partitions 0-63, odd on 64-127).
  * exp split across TWO engines (the ScalarE LUT is the bottleneck
    resource at 1 elem/cycle/lane):
      - ScalarE: activation(Exp) with fused accum_out on 1536 of each
        tile's 2048 columns.
      - VectorE: Schraudolph fp16 exp on the last 512 columns (its own
        PSUM bank so the engines never share a bank): a tensor_scalar
        affine fp32->int16 written through a bitcast view of the fp16 et
        tile; code = round(S*scale*log2e*1024) + (15*1024 - 45). Applied
        to ~1/4 of the elements -> ~8e-3 output rel err (gate 2e-2).
        The very last tile-pair stays ScalarE-only so the kernel tail
        has no VectorE exp chain.
      - DVE-part colsum: tensor_scalar copy with accum_out over those
        512 columns; partials combined with the ACT accumulators.
  * PE HAM clock-gate management (worth ~30us): a 20-matmul full-array
    warm-up burst at kernel start plus 8-matmul re-warm bursts in each
    bh-boundary bubble (the out banks are free right after evacuation).
  * normalization folded into V: vp = V * (1/colsum) per k2-tile (fp16).
  * mm2: out^T += vp.T @ et with 2x64-col PE col-group pairing, 6 tiles
    deep behind the exp stream (10 et buffers), drained to depth 2 near
    the end of the last bh.

Sharding: batch*heads = 32 pairs, 4 per core across 8 cores (head/data
parallel, no cross-core communication).
"""

import sys

import numpy as np

if "/opt/trn_rl_repo" not in sys.path:
    sys.path.insert(0, "/opt/trn_rl_repo")

import concourse.tile as tile  # noqa: E402
from concourse import bacc, mybir  # noqa: E402
from concourse.bass_utils import run_bass_kernel_spmd  # noqa: E402

P = 128
D = 64
SCALE = 1.0 / (D ** 0.5)
N_CORES = 8

F32 = mybir.dt.float32
F16 = mybir.dt.float16
I16 = mybir.dt.int16

LOG2E = 1.4426950408889634
# Schraudolph fp16: code = trunc(S * AFF_A + AFF_B); bitcast fp16 ~ exp(S*SCALE)
AFF_A = SCALE * LOG2E * 1024.0
AFF_B = 15.0 * 1024.0 - 45.0  # HW convert rounds to nearest (probe-verified)

# Columns per TILE handled by the DVE Schraudolph path (rest on ACT).
# Must be a whole number of PSUM banks (512) so ScalarE and VectorE never
# read the same PSUM bank concurrently.
W_DVE = 512


def build_attention_nc(BH: int, T: int, debug: bool = False):
    """Per-core Bass module. See module docstring for layouts."""
    assert T % 1024 == 0 and T % P == 0
    KT_TILES = T // P
    CHUNK = 1024  # PSUM score-chunk (2 banks)

    nc = bacc.Bacc("TRN2", target_bir_lowering=False, debug=debug)

    qt = nc.dram_tensor("qt", [BH, 2 * D, T], F16, kind="ExternalInput").ap()
    kt = nc.dram_tensor("kt", [BH, 2 * D, T // 2], F16, kind="ExternalInput").ap()
    v = nc.dram_tensor("v", [BH, P, T // P, D], F32, kind="ExternalInput").ap()
    out = nc.dram_tensor("out", [BH, D, T], F32, kind="ExternalOutput").ap()

    with tile.TileContext(nc) as tc:
        with (
            tc.tile_pool(name="ins", bufs=1) as ins_pool,
            tc.tile_pool(name="et", bufs=10) as et_pool,
            tc.tile_pool(name="small", bufs=8) as small_pool,
            tc.tile_pool(name="osb", bufs=2) as osb_pool,
            tc.tile_pool(name="spsum", bufs=3, space="PSUM") as s_pool,
            tc.tile_pool(name="opsum", bufs=1, space="PSUM") as o_pool,
        ):
            qt_sb = ins_pool.tile([2 * D, BH, T], F16, tag="qt_sb")
            kt_sb = ins_pool.tile([2 * D, BH, T // 2], F16, tag="kt_sb")
            v_sb = ins_pool.tile([P, BH, KT_TILES, D], F32, tag="v_sb")
            # scratch sink for the DVE-part colsum pass
            sink = ins_pool.tile([P, W_DVE], F16, tag="sink")
            # Warm the ACT exp table during input DMAs.
            warm = small_pool.tile([P, 1], F32, tag="warm")
            nc.vector.memset(warm[:], 0.0)
            nc.scalar.activation(
                warm[:], warm[:], mybir.ActivationFunctionType.Exp
            )
            # Warm the PE HAM clock gate during the input DMAs: ~7us of
            # dummy matmuls (no DMA dependency) cross the 4096-cycle SHORT
            # window so the real mm1 stream starts at 2.4 GHz.
            wdum = ins_pool.tile([P, 512], F16, tag="wdum")
            nc.vector.memset(wdum[:], 0.0)
            warm_ps = o_pool.tile([2 * D, T // 2], F32, tag="out_ps")
            for _ in range(20):
                nc.tensor.matmul(
                    warm_ps[:, 0:512],
                    lhsT=wdum[:, 0:128],
                    rhs=wdum[:],
                    start=True,
                    stop=True,
                )

            for bh in range(BH):
                nc.sync.dma_start(qt_sb[:, bh, :], qt[bh])
                nc.sync.dma_start(kt_sb[:, bh, :], kt[bh])
                nc.sync.dma_start(v_sb[:, bh], v[bh])

            def emit_mm2(out_ps, vp, et, t):
                for c in range(0, T, 512):
                    half = c // (T // 2)
                    qh = c % (T // 2)
                    nc.tensor.matmul(
                        out_ps[half * D:(half + 1) * D, qh:qh + 512],
                        lhsT=vp[:],
                        rhs=et[:, c:c + 512],
                        start=(t == 0),
                        stop=(t == KT_TILES - 1),
                        skip_group_check=True,
                    )

            def emit_norm_vp(bh, t, et, partial_sums):
                """Combine colsum partials + reciprocal + scale V."""
                while len(partial_sums) > 1:
                    stot = small_pool.tile([P, 1], F32, tag="stot")
                    nc.vector.tensor_add(
                        stot[:], partial_sums[0][:], partial_sums[1][:]
                    )
                    partial_sums = [stot] + partial_sums[2:]
                rec = small_pool.tile([P, 1], F32, tag="rec")
                nc.vector.reciprocal(rec[:], partial_sums[0][:])
                vp = small_pool.tile([P, D], F16, tag="vp")
                nc.vector.tensor_scalar_mul(vp[:], v_sb[:, bh, t, :], rec[:])
                return vp

            def evacuate(bh, out_ps):
                osb = osb_pool.tile([2 * D, T // 2], F32, tag="osb")
                nc.vector.tensor_copy(osb[:], out_ps[:])
                nc.sync.dma_start(out[bh][:, 0:T // 2], osb[0:D])
                nc.sync.dma_start(out[bh][:, T // 2:T], osb[D:2 * D])
                if bh < BH - 1:
                    # keep the PE HAM warm through the boundary bubble: the
                    # out banks are free between the copy above and the
                    # next bh's first accumulating matmul
                    rw = o_pool.tile([2 * D, T // 2], F32, tag="out_ps")
                    for _ in range(8):
                        nc.tensor.matmul(
                            rw[:, 0:512],
                            lhsT=wdum[:, 0:128],
                            rhs=wdum[:],
                            start=True,
                            stop=True,
                        )

            def pop_mm2(pending):
                bh_, out_ps_, vp_, et_, t_ = pending.pop(0)
                emit_mm2(out_ps_, vp_, et_, t_)
                if t_ == KT_TILES - 1:
                    evacuate(bh_, out_ps_)

            pending_mm2 = []
            for bh in range(BH):
                out_ps = o_pool.tile([2 * D, T // 2], F32, tag="out_ps")
                for j in range(KT_TILES // 2):
                    tA, tB = 2 * j, 2 * j + 1
                    etA = et_pool.tile([P, T], F16, tag="et", name="etA")
                    etB = et_pool.tile([P, T], F16, tag="et", name="etB")
                    lhsA = kt_sb[0:D, bh, j * P:(j + 1) * P]
                    lhsB = kt_sb[D:2 * D, bh, j * P:(j + 1) * P]
                    psums = {tA: [], tB: []}
                    for q0 in range(0, T, CHUNK):
                        spA = s_pool.tile([P, CHUNK], F32, tag="sp",
                                          name="spA")
                        spB = s_pool.tile([P, CHUNK], F32, tag="sp",
                                          name="spB")
                        for c in range(0, CHUNK, 512):
                            nc.tensor.matmul(
                                spA[:, c:c + 512],
                                lhsT=lhsA,
                                rhs=qt_sb[0:D, bh, q0 + c:q0 + c + 512],
                                start=True,
                                stop=True,
                            )
                            nc.tensor.matmul(
                                spB[:, c:c + 512],
                                lhsT=lhsB,
                                rhs=qt_sb[D:2 * D, bh, q0 + c:q0 + c + 512],
                                start=True,
                                stop=True,
                            )
                        # exp: last W_DVE cols of the LAST chunk go to the
                        # DVE Schraudolph path (own PSUM bank); rest -> ACT
                        # with fused accum (colsum partials for free).
                        last = q0 + CHUNK == T
                        w = W_DVE
                        if bh == BH - 1 and j == KT_TILES // 2 - 1:
                            w = 0  # keep the tail chain ScalarE-only
                        na = CHUNK - w if last else CHUNK
                        for t, et, sp in ((tA, etA, spA), (tB, etB, spB)):
                            acc = small_pool.tile([P, 1], F32, tag="acc")
                            nc.scalar.activation(
                                et[:, q0:q0 + na],
                                sp[:, 0:na],
                                mybir.ActivationFunctionType.Exp,
                                scale=SCALE,
                                accum_out=acc[:],
                            )
                            psums[t].append(acc)
                            if last and w:
                                nc.vector.tensor_scalar(
                                    et[:, q0 + na:q0 + CHUNK].bitcast(I16),
                                    sp[:, na:CHUNK],
                                    AFF_A,
                                    AFF_B,
                                    mybir.AluOpType.mult,
                                    mybir.AluOpType.add,
                                )
                                dacc = small_pool.tile([P, 1], F32, tag="dacc")
                                nc.vector.tensor_scalar(
                                    sink[:, 0:W_DVE],
                                    et[:, q0 + na:q0 + CHUNK],
                                    1.0,
                                    0.0,
                                    mybir.AluOpType.mult,
                                    mybir.AluOpType.add,
                                    accum_out=dacc[:],
                                )
                                psums[t].append(dacc)
                    for t, et in ((tA, etA), (tB, etB)):
                        vp = emit_norm_vp(bh, t, et, psums[t])
                        pending_mm2.append((bh, out_ps, vp, et, t))
                    # deep pipeline in steady state; drain eagerly near the
                    # end so the final mm2/evacuation tail is short
                    depth = 6 if (bh < BH - 1 or j < KT_TILES // 2 - 4) else 2
                    while len(pending_mm2) > depth:
                        pop_mm2(pending_mm2)
            while pending_mm2:
                pop_mm2(pending_mm2)

    nc.compile()
    return nc


_NC_CACHE: dict = {}

TRACE = False
LAST_RESULTS = None


def _get_nc(BH: int, T: int):
    key = (BH, T)
    if key not in _NC_CACHE:
        _NC_CACHE[key] = build_attention_nc(BH, T)
    return _NC_CACHE[key]


def _reference_numpy(Q, K, V, padding_mask, isCausal):
    """Fallback exactly mirroring reference.py (never hit for spec inputs)."""
    Q = Q.astype(np.float64)
    K = K.astype(np.float64)
    V = V.astype(np.float64)
    scores = np.einsum("bhqd,bhkd->bhqk", Q, K) * SCALE
    T1 = scores.shape[2]
    mask = padding_mask[:, None, :, :].astype(np.float64)
    if isCausal:
        mask = mask * np.tril(np.ones((T1, T1)))
    scores = np.where(mask == 0, -np.inf, scores)
    m = np.max(scores, axis=2, keepdims=True)
    e = np.exp(scores - m)
    attn = e / np.sum(e, axis=2, keepdims=True)
    return np.einsum("bhqk,bhkd->bhqd", attn, V).astype(np.float32)


def kernel(Q, K, V, padding_mask, isCausal, **_unused):
    Q = np.asarray(Q, dtype=np.float32)
    K = np.asarray(K, dtype=np.float32)
    V = np.asarray(V, dtype=np.float32)
    padding_mask = np.asarray(padding_mask)
    causal = int(np.asarray(isCausal))

    B, H, T, Dd = Q.shape
    assert Dd == D
    if causal != 0 or padding_mask.min() != 1.0 or padding_mask.max() != 1.0:
        return _reference_numpy(Q, K, V, padding_mask, causal)

    BHT = B * H
    assert BHT % N_CORES == 0
    BH = BHT // N_CORES

    nc = _get_nc(BH, T)

    Qf = Q.reshape(BHT, T, D)
    Kf = K.reshape(BHT, T, D)
    Vf = V.reshape(BHT, T, D)

    QT = Qf.transpose(0, 2, 1).astype(np.float16)
    qt_all = np.ascontiguousarray(np.concatenate([QT, QT], axis=1))
    KT = Kf.transpose(0, 2, 1).astype(np.float16)
    KT4 = KT.reshape(BHT, D, T // 128, 128)
    kt_all = np.ascontiguousarray(
        np.concatenate(
            [
                KT4[:, :, 0::2, :].reshape(BHT, D, T // 2),
                KT4[:, :, 1::2, :].reshape(BHT, D, T // 2),
            ],
            axis=1,
        )
    )
    v_all = np.ascontiguousarray(
        Vf.reshape(BHT, T // P, P, D).transpose(0, 2, 1, 3)
    )

    in_maps = []
    for c in range(N_CORES):
        sl = slice(c * BH, (c + 1) * BH)
        in_maps.append(
            {
                "qt": np.ascontiguousarray(qt_all[sl]),
                "kt": np.ascontiguousarray(kt_all[sl]),
                "v": np.ascontiguousarray(v_all[sl]),
            }
        )

    res = None
    last_err = None
    for attempt in range(3):
        try:
            res = run_bass_kernel_spmd(
                nc, in_maps, core_ids=list(range(N_CORES)), trace=TRACE
            )
            break
        except Exception as e:
            last_err = e
            import time as _time

            _time.sleep(2.0)
    if res is None:
        raise last_err
    global LAST_RESULTS
    LAST_RESULTS = res

    outs = [res.results[c]["out"] for c in range(N_CORES)]
    out_all = np.concatenate(outs, axis=0)
    out = out_all.transpose(0, 2, 1).reshape(B, H, T, D)
    return np.ascontiguousarray(out).astype(np.float32)
